# revision 15
# baseline (speedup 1.0000x reference)
"""Binarized ResNet Bottleneck block (dense_cnn) on 8 TRN2 NeuronCores.

Math: with inference BN folded to z*s + c (s = g*rsqrt(v+eps) > 0, c = b - m*s),
binarize(htanh(bn(z))) == sign(z*s + c).  Normalizing the final sign by s3
(instead of ssc) makes conv3 an EXACT +-1 integer GEMM:

  y1  = sign(conv1(x)  + c1/s1)                        # 1x1, 256->128
  y2  = sign(conv2(y1) + c2/s2)                        # 3x3 stride 2, pad 1
  out = sign(S + q'*SC + c'')                          # S = conv3(y2) int,
        q' = ssc/s3, c'' = (c3+csc)/s3                 # SC = convsc(x)

All conv weights binarize to +-1, so each conv is a GEMM with pixels on the
PE free dim.  x is pre-scaled by 1024 and split into fp16 hi + fp8e4 lo
(residual ~2^-15 relative; ~900 of 12.8M output bits flip, rel-err 0.017
vs the 0.02 budget).  The lo plane stores the two 128-channel k-tiles as
interleaved byte pairs, so one fp8 DoubleRow matmul contracts K=256 at the
cost of one K=128 matmul: conv1 runs hi-kt0 + hi-kt1 + one DR-lo per pixel
tile.  y1/y2 are exactly +-1 in fp8e4; conv2 pairs taps in DoubleRow MMs
((dy,0)+(dy,1) via adjacent bytes, (0,2)+(1,2) via adjacent rows) - 4 DR +
1 single instead of 9 MMs.  The shortcut reads the stride-2 quadrant of the
same x planes: 2 fp16-hi MMs (w = +-2^-10 exact) + 1 DR-lo (w = +-2^-10
e5m2, exact powers of 2), conv3's integer S in its own PSUM bank, and the
otherwise-idle VectorE applies u = (SC*q' + c'') + S with ScalarE's Sign.

Startup: x/weight first chunks are the only traffic on the sync HWDGE queue
(scalar stays free for the Sign activations - a DMA ahead of them blocks
the strict-FIFO queue on ring credits), and a short zero-matmul warmup in a
stage-2 PSUM bank opens the PE HAM clock gate (1.2 -> 2.4 GHz) while the
first chunks land.
"""

import numpy as np
import ml_dtypes

import concourse.bass as bass
import concourse.tile as tile
from concourse import bacc, mybir
from concourse.bass_utils import run_bass_kernel_spmd

F16 = mybir.dt.float16
F32 = mybir.dt.float32
E4 = mybir.dt.float8e4
E5 = mybir.dt.float8e5
AF = mybir.ActivationFunctionType
ALU = mybir.AluOpType
DRM = mybir.MatmulPerfMode.DoubleRow
E4NP = ml_dtypes.float8_e4m3fn
E5NP = ml_dtypes.float8_e5m2

EPS = 1e-5
NB, CIN, H, W = 32, 256, 56, 56
PLANES, OUTP = 128, 512
NCORES = 8
NPC = NB // NCORES              # samples per core
HW1 = H * W                     # 3136
WP = W + 2                      # 58 (padded row length)
HP = H + 2
HO = WO = 28
HWO = HO * WO                   # 784
PT1 = 392                       # conv1 pixel tile = 7 rows of 56
NPT1 = HW1 // PT1               # 8
PT3 = 392                       # stage2/3 pixel tile = 14 out rows of 28
SCALE = 1024.0                  # x pre-scale (power of two)

# DMA chunk plan (pixels): fine for sample 0 so conv1 starts early
CHUNKS = {0: [(0, PT1), (PT1, PT1), (2 * PT1, 2 * PT1), (4 * PT1, 4 * PT1)]}
for _n in range(1, NPC):
    CHUNKS[_n] = [(0, 4 * PT1), (4 * PT1, 4 * PT1)]

# wts16 column layout (fp16, 128 partitions = contraction dim)
_B1 = 0            # 2 ktiles x 128: conv1 hi +-1
_S3 = 256          # 4 oc x 128: conv3 +-1
_SCHI = 768        # 2 ktiles x 512: shortcut hi +-2^-10
_W16COLS = 1792

# wts8 column layout (fp8e4)
_B2P = 0           # 4 x [2 x 128]: conv2 tap pairs
_B2S = 1024        # 128: conv2 single tap (2,2)
_B1L = 1152        # [2 x 128]: conv1 lo +-1 (kt pair)
_W8COLS = 1408

# wsc8 (fp8e5): 4 oc x [2 kt x 128]: shortcut lo +-2^-10
_WSCCOLS = 1024

# bias cols ([128, 10] f32):
#   0: (c1/s1)*SCALE   1: c2/s2   2+oc: q' = ssc/s3   6+oc: c'' = (c3+csc)/s3

# conv2 tap pairs (dy, dx): three same-row pairs + one same-col pair + single
PAIRS = [((0, 0), (0, 1)), ((1, 0), (1, 1)), ((2, 0), (2, 1)), ((0, 2), (1, 2))]
SINGLE = (2, 2)


def build_bass():
    nc = bacc.Bacc("TRN2", target_bir_lowering=False, debug=False)
    nxh = NPC * 2 * 128 * HW1
    xhi_d = nc.dram_tensor("xhi", [nxh], F16, kind="ExternalInput")
    xlo8_d = nc.dram_tensor("xlo8", [nxh], E4, kind="ExternalInput")
    w16_d = nc.dram_tensor("w16", [128, _W16COLS], F16, kind="ExternalInput")
    w8_d = nc.dram_tensor("w8", [128, _W8COLS], E4, kind="ExternalInput")
    wsc8_d = nc.dram_tensor("wsc8", [128, _WSCCOLS], E5, kind="ExternalInput")
    bias_d = nc.dram_tensor("bias", [128, 10], F32, kind="ExternalInput")
    out_d = nc.dram_tensor("out", [NPC * 4 * 2 * 128, PT3], E4, kind="ExternalOutput")
    warm_d = nc.dram_tensor("warm", [128, 8], F32, kind="ExternalOutput")

    with tile.TileContext(nc) as tc:
        import contextlib

        with contextlib.ExitStack() as ctx:
            const = ctx.enter_context(tc.tile_pool(name="const", bufs=1))
            xpool = ctx.enter_context(tc.tile_pool(name="x", bufs=1))
            ypool = ctx.enter_context(tc.tile_pool(name="y", bufs=1))
            opool = ctx.enter_context(tc.tile_pool(name="o", bufs=16))
            upool = ctx.enter_context(tc.tile_pool(name="u", bufs=4))
            p1pool = ctx.enter_context(tc.tile_pool(name="p1", bufs=2, space="PSUM"))
            p2pool = ctx.enter_context(tc.tile_pool(name="p2", bufs=2, space="PSUM"))
            pscpool = ctx.enter_context(tc.tile_pool(name="psc", bufs=2, space="PSUM"))
            ps3pool = ctx.enter_context(tc.tile_pool(name="ps3", bufs=2, space="PSUM"))

            # conv1's fp16 weight block ships first on sync; the rest of the
            # fp16 weights ride scalar once, ahead of all ACTs
            w16 = const.tile([128, _W16COLS], F16, tag="w16")
            bias = const.tile([128, 10], F32, tag="bias")
            w8 = const.tile([128, _W8COLS], E4, tag="w8")
            wsc8 = const.tile([128, _WSCCOLS], E5, tag="wsc8")
            nc.scalar.dma_start(bias[:], bias_d.ap())
            nc.scalar.dma_start(w8[:], w8_d.ap())
            nc.scalar.dma_start(wsc8[:], wsc8_d.ap())

            # PE prewarm in a stage-2 PSUM bank (stage-1 banks stay free for
            # the first real matmuls).  The input tile is never written -
            # garbage values are fine and skipping the memset removes the
            # cross-engine dependency, so the PE starts the moment its queue
            # opens.  Escape chain prevents DCE.
            warm = const.tile([128, 512], F16, tag="warm")
            nc.vector.memset(warm[:], 0.0)
            for r in range(8):
                pw = p2pool.tile([128, 512], F32, tag="p2", name=f"warm{r}")
                nc.tensor.matmul(
                    pw[:], warm[:, 0:128], warm[:], start=True, stop=True
                )
            for r in range(24):
                pw = p2pool.tile([128, 512], F32, tag="p2", name=f"warmb{r}")
                nc.tensor.matmul(
                    pw[:, 0:128], warm[:, 0:128], warm[:, 0:128],
                    start=True, stop=True,
                )
            wout = const.tile([128, 8], F32, tag="wout")
            nc.vector.tensor_copy(wout[:], pw[:, 0:8])

            xhi = {}
            xlo8 = {}
            for n in range(NPC):
                for kt in range(2):
                    xhi[n, kt] = xpool.tile(
                        [128, HW1], F16, tag=f"xhi{n}{kt}", name=f"xhi{n}{kt}"
                    )
                xlo8[n] = xpool.tile(
                    [128, 2 * HW1], E4, tag=f"xlo8{n}", name=f"xlo8{n}"
                )

            # x DMAs: DRAM is chunk-contiguous in emission order.  The very
            # first chunk's three pieces land in parallel on sync/scalar/
            # gpsimd; everything after streams on sync (which carries nothing
            # else until the tail).  w16's conv1 block follows immediately;
            # the rest of w16 rides scalar once, ahead of all ACTs.
            offh = 0
            offl = 0
            first = True
            for n in range(NPC):
                for p0, w in CHUNKS[n]:
                    for kt in range(2):
                        span = 128 * w
                        src_hi = xhi_d.ap()[offh : offh + span].rearrange(
                            "(p w) -> p w", w=w
                        )
                        eng = nc.scalar if (first and kt == 1) else nc.sync
                        eng.dma_start(xhi[n, kt][:, p0 : p0 + w], src_hi)
                        offh += span
                    span = 128 * 2 * w
                    src_lo = xlo8_d.ap()[offl : offl + span].rearrange(
                        "(p w) -> p w", w=2 * w
                    )
                    (nc.scalar if first else nc.sync).dma_start(
                        xlo8[n][:, 2 * p0 : 2 * (p0 + w)], src_lo
                    )
                    offl += span
                    if first:
                        nc.sync.dma_start(w16[:, 0:256], w16_d.ap()[:, 0:256])
                        nc.scalar.dma_start(
                            w16[:, 256:_W16COLS], w16_d.ap()[:, 256:_W16COLS]
                        )
                        first = False
            nc.sync.dma_start(warm_d.ap(), wout[:])

            y1 = {}
            y2 = {}
            for n in range(NPC):
                y1[n] = ypool.tile([128, HP * WP], E4, tag=f"y1_{n}", name=f"y1_{n}")
                y2[n] = ypool.tile([128, HWO], E4, tag=f"y2_{n}", name=f"y2_{n}")

            for n in range(NPC):
                v1 = y1[n][:].rearrange("p (h w) -> p h w", w=WP)
                nc.vector.memset(v1[:, 0:1, :], 0.0)
                nc.vector.memset(v1[:, HP - 1 : HP, :], 0.0)
                nc.vector.memset(v1[:, 1 : HP - 1, 0:1], 0.0)
                nc.vector.memset(v1[:, 1 : HP - 1, WP - 1 : WP], 0.0)

            def stage1(n, pts=range(NPT1)):
                v1 = y1[n][:].rearrange("p (h w) -> p h w", w=WP)
                for pt in pts:
                    p1 = p1pool.tile([128, PT1], F32, tag="p1")
                    ps = slice(pt * PT1, (pt + 1) * PT1)
                    for kt in range(2):
                        nc.tensor.matmul(
                            p1[:],
                            w16[:, _B1 + kt * 128 : _B1 + kt * 128 + 128],
                            xhi[n, kt][:, ps],
                            start=(kt == 0),
                            stop=False,
                        )
                    lo_rhs = (
                        xlo8[n][:, 2 * pt * PT1 : 2 * (pt + 1) * PT1]
                        .rearrange("p (w two) -> p two w", two=2)
                    )
                    nc.tensor.matmul(
                        p1[:],
                        w8[:, _B1L : _B1L + 256].rearrange(
                            "p (two m) -> p two m", two=2
                        ),
                        lo_rhs,
                        start=False,
                        stop=True,
                        perf_mode=DRM,
                    )
                    nc.scalar.activation(
                        v1[:, 7 * pt + 1 : 7 * pt + 8, 1 : 1 + W],
                        p1[:].rearrange("p (h w) -> p h w", w=W),
                        AF.Sign,
                        bias=bias[:, 0:1],
                        scale=1.0,
                    )

            def stage2(n, ht):
                v1 = y1[n][:].rearrange("p (h w) -> p h w", w=WP)
                p2 = p2pool.tile([128, PT3], F32, tag="p2", name=f"p2_{n}_{ht}")
                for i, ((dy0, dx0), (dy1, dx1)) in enumerate(PAIRS):
                    if dy0 == dy1:
                        # same row: adjacent-byte pairs (dx0, dx0+1)
                        rows = v1[:, 28 * ht + dy0 : 28 * ht + dy0 + 28 : 2, :]
                        pair = rows[:, :, dx0 : dx0 + 56].rearrange(
                            "p h (w two) -> p two h w", two=2
                        )
                    else:
                        # same col: adjacent-row pairs (dy0, dy0+1)
                        rows = v1[:, 28 * ht + dy0 : 28 * ht + dy0 + 28, :]
                        pair = rows.rearrange("p (h two) w -> p two h w", two=2)[
                            :, :, :, dx0 : dx0 + 56 : 2
                        ]
                    nc.tensor.matmul(
                        p2[:],
                        w8[:, _B2P + i * 256 : _B2P + (i + 1) * 256].rearrange(
                            "p (two m) -> p two m", two=2
                        ),
                        pair,
                        start=(i == 0),
                        stop=False,
                        perf_mode=DRM,
                    )
                dy, dx = SINGLE
                nc.tensor.matmul(
                    p2[:],
                    w8[:, _B2S : _B2S + 128],
                    v1[:, 28 * ht + dy : 28 * ht + dy + 28 : 2, dx : dx + 56 : 2],
                    start=False,
                    stop=True,
                )
                nc.scalar.activation(
                    y2[n][:, ht * PT3 : (ht + 1) * PT3],
                    p2[:],
                    AF.Sign,
                    bias=bias[:, 1:2],
                    scale=1.0,
                )

            def stage3(n, ht):
                yslice = y2[n][:, ht * PT3 : (ht + 1) * PT3]
                # shortcut lo: stride-2 quadrant of the full fp8 plane,
                # kt byte-pairs: [p, 2(s=1), 14 rows, 28 cols]
                vlo = xlo8[n][:].rearrange("p (h w two) -> p two h w", w=W, two=2)
                lo_rhs = vlo[:, :, 28 * ht : 28 * ht + 28 : 2, 0:56:2]
                for oc in range(4):
                    psc = pscpool.tile([128, PT3], F32, tag="psc")
                    for kt in range(2):
                        rhs = (
                            xhi[n, kt][:]
                            .rearrange("p (h w) -> p h w", w=W)
                            [:, 28 * ht : 28 * ht + 28 : 2, 0:56:2]
                        )
                        nc.tensor.matmul(
                            psc[:],
                            w16[
                                :,
                                _SCHI + kt * 512 + oc * 128 : _SCHI
                                + kt * 512
                                + oc * 128
                                + 128,
                            ],
                            rhs,
                            start=(kt == 0),
                            stop=False,
                        )
                    nc.tensor.matmul(
                        psc[:],
                        wsc8[:, oc * 256 : (oc + 1) * 256].rearrange(
                            "p (two m) -> p two m", two=2
                        ),
                        lo_rhs,
                        start=False,
                        stop=True,
                        perf_mode=DRM,
                    )
                    ps3 = ps3pool.tile([128, PT3], F32, tag="ps3")
                    nc.tensor.matmul(
                        ps3[:],
                        w16[:, _S3 + oc * 128 : _S3 + oc * 128 + 128],
                        yslice,
                        start=True,
                        stop=True,
                    )
                    u = upool.tile([128, PT3], F32, tag="u")
                    nc.vector.tensor_scalar(
                        u[:],
                        psc[:],
                        bias[:, 2 + oc : 3 + oc],
                        bias[:, 6 + oc : 7 + oc],
                        ALU.mult,
                        ALU.add,
                    )
                    nc.vector.tensor_tensor(u[:], u[:], ps3[:], ALU.add)
                    ot = opool.tile([128, PT3], E4, tag="ot")
                    nc.scalar.activation(ot[:], u[:], AF.Sign, bias=0.0, scale=1.0)
                    nc.sync.dma_start(
                        out_d.ap()[
                            ((n * 4 + oc) * 2 + ht) * 128 : ((n * 4 + oc) * 2 + ht + 1)
                            * 128,
                            :,
                        ],
                        ot[:],
                    )

            for n in range(NPC):
                if n == 0:
                    stage1(n, range(5))
                    stage2(n, 0)
                    stage1(n, range(5, NPT1))
                    stage2(n, 1)
                else:
                    stage1(n)
                    stage2(n, 0)
                    stage2(n, 1)
                stage3(n, 0)
                stage3(n, 1)

    nc.compile()
    return nc


def _prep_inputs(x, W1, W2, W3, Wsc, g1, b1, m1, v1, g2, b2, m2, v2,
                 g3, b3, m3, v3, gs, bs, ms, vs):
    f32 = np.float32

    def sgn(w):
        return np.where(w >= 0, 1.0, -1.0).astype(f32)

    def fold(g, b, m, v):
        s = (g / np.sqrt(v + EPS)).astype(f32)
        return s, (b - m * s).astype(f32)

    s1, c1 = fold(g1, b1, m1, v1)
    s2, c2 = fold(g2, b2, m2, v2)
    s3, c3 = fold(g3, b3, m3, v3)
    ssc, csc = fold(gs, bs, ms, vs)

    w16 = np.zeros((128, _W16COLS), np.float16)
    b1t = sgn(W1[:, :, 0, 0]).T                     # [256, 128]
    w16[:, _B1 : _B1 + 128] = b1t[:128]
    w16[:, _B1 + 128 : _B1 + 256] = b1t[128:]
    w3t = sgn(W3[:, :, 0, 0]).T                     # [128, 512]
    w16[:, _S3 : _S3 + 512] = w3t
    wsct = sgn(Wsc[:, :, 0, 0]).T * f32(1.0 / SCALE)  # [256, 512], +-2^-10
    w16[:, _SCHI : _SCHI + 512] = wsct[:128]
    w16[:, _SCHI + 512 : _SCHI + 1024] = wsct[128:]

    w8 = np.zeros((128, _W8COLS), E4NP)
    b2v = sgn(W2)                                   # [128, 128, 3, 3]
    for i, ((dy0, dx0), (dy1, dx1)) in enumerate(PAIRS):
        w8[:, _B2P + i * 256 : _B2P + i * 256 + 128] = b2v[:, :, dy0, dx0].T.astype(
            E4NP
        )
        w8[:, _B2P + i * 256 + 128 : _B2P + (i + 1) * 256] = (
            b2v[:, :, dy1, dx1].T.astype(E4NP)
        )
    w8[:, _B2S : _B2S + 128] = b2v[:, :, SINGLE[0], SINGLE[1]].T.astype(E4NP)
    w8[:, _B1L : _B1L + 128] = b1t[:128].astype(E4NP)
    w8[:, _B1L + 128 : _B1L + 256] = b1t[128:].astype(E4NP)

    wsc8 = np.zeros((128, _WSCCOLS), E5NP)
    for oc in range(4):
        for kt in range(2):
            blk = wsct[kt * 128 : (kt + 1) * 128, oc * 128 : (oc + 1) * 128]
            wsc8[:, oc * 256 + kt * 128 : oc * 256 + (kt + 1) * 128] = blk.astype(
                E5NP
            )

    bias = np.zeros((128, 10), f32)
    bias[:, 0] = (c1 / s1) * f32(SCALE)
    bias[:, 1] = c2 / s2
    bias[:, 2:6] = (ssc / s3).reshape(4, 128).T
    bias[:, 6:10] = ((c3 + csc) / s3).reshape(4, 128).T

    xs = (x.astype(f32) * f32(SCALE)).reshape(NB, 2, 128, HW1)
    xhi = xs.astype(np.float16)
    xlo_f = xs - xhi.astype(f32)
    # kt byte-pairs: [NB, 128, HW1, 2]
    xlo8 = xlo_f.transpose(0, 2, 3, 1).astype(E4NP)

    # chunk-contiguous per-core flat layout matching build_bass emission order
    xhic = []
    xloc = []
    for c in range(NCORES):
        ph = []
        pl = []
        for n in range(NPC):
            gh = xhi[c * NPC + n]              # [2, 128, HW1]
            gl = xlo8[c * NPC + n]             # [128, HW1, 2]
            for p0, w in CHUNKS[n]:
                for kt in range(2):
                    ph.append(gh[kt, :, p0 : p0 + w].reshape(-1))
                pl.append(gl[:, p0 : p0 + w, :].reshape(-1))
        xhic.append(np.concatenate(ph))
        xloc.append(np.concatenate(pl))
    return xhic, xloc, w16, w8, wsc8, bias


_NC_CACHE = []


def _assemble(res_results):
    outs = []
    for r in res_results:
        o = np.asarray(r["out"]).astype(np.float32).reshape(NPC, 4, 2, 128, PT3)
        o = o.transpose(0, 1, 3, 2, 4).reshape(NPC, OUTP, HO, WO)
        outs.append(o)
    return np.concatenate(outs, axis=0).astype(np.float32)


def make_in_maps(inputs):
    xhi, xlo8, w16, w8, wsc8, bias = _prep_inputs(**inputs)
    return [
        {
            "xhi": xhi[c],
            "xlo8": xlo8[c],
            "w16": w16,
            "w8": w8,
            "wsc8": wsc8,
            "bias": bias,
        }
        for c in range(NCORES)
    ]


def kernel(**inputs):
    inputs = {k: np.asarray(v) for k, v in inputs.items()}
    in_maps = make_in_maps(inputs)
    if not _NC_CACHE:
        _NC_CACHE.append(build_bass())
    nc = _NC_CACHE[0]
    res = run_bass_kernel_spmd(nc, in_maps, core_ids=list(range(NCORES)))
    return _assemble(res.results)


# revision 16
# speedup vs baseline: 1.0494x; 1.0494x over previous
"""Binarized ResNet Bottleneck block (dense_cnn) on 8 TRN2 NeuronCores.

Math: with inference BN folded to z*s + c (s = g*rsqrt(v+eps) > 0, c = b - m*s),
binarize(htanh(bn(z))) == sign(z*s + c).  Normalizing the final sign by s3
(instead of ssc) makes conv3 an EXACT +-1 integer GEMM:

  y1  = sign(conv1(x)  + c1/s1)                        # 1x1, 256->128
  y2  = sign(conv2(y1) + c2/s2)                        # 3x3 stride 2, pad 1
  out = sign(S + q'*SC + c'')                          # S = conv3(y2) int,
        q' = ssc/s3, c'' = (c3+csc)/s3                 # SC = convsc(x)

All conv weights binarize to +-1, so each conv is a GEMM with pixels on the
PE free dim.  x is pre-scaled by 1024 and split into fp16 hi + fp8e4 lo
(residual ~2^-15 relative; ~900 of 12.8M output bits flip, rel-err 0.017
vs the 0.02 budget).  The lo plane stores the two 128-channel k-tiles as
interleaved byte pairs, so one fp8 DoubleRow matmul contracts K=256 at the
cost of one K=128 matmul: conv1 runs hi-kt0 + hi-kt1 + one DR-lo per pixel
tile.  y1/y2 are exactly +-1 in fp8e4; conv2 pairs taps in DoubleRow MMs
((dy,0)+(dy,1) via adjacent bytes, (0,2)+(1,2) via adjacent rows) - 4 DR +
1 single instead of 9 MMs.  The shortcut reads the stride-2 quadrant of the
same x planes: 2 fp16-hi MMs (w = +-2^-10 exact) + 1 DR-lo (w = +-2^-10
e5m2, exact powers of 2), conv3's integer S in its own PSUM bank, and the
otherwise-idle VectorE applies u = (SC*q' + c'') + S with ScalarE's Sign.

Startup: x/weight first chunks are the only traffic on the sync HWDGE queue
(scalar stays free for the Sign activations - a DMA ahead of them blocks
the strict-FIFO queue on ring credits), and a short zero-matmul warmup in a
stage-2 PSUM bank opens the PE HAM clock gate (1.2 -> 2.4 GHz) while the
first chunks land.
"""

import numpy as np
import ml_dtypes

import concourse.bass as bass
import concourse.tile as tile
from concourse import bacc, mybir
from concourse.bass_utils import run_bass_kernel_spmd

F16 = mybir.dt.float16
F32 = mybir.dt.float32
E4 = mybir.dt.float8e4
E5 = mybir.dt.float8e5
AF = mybir.ActivationFunctionType
ALU = mybir.AluOpType
DRM = mybir.MatmulPerfMode.DoubleRow
E4NP = ml_dtypes.float8_e4m3fn
E5NP = ml_dtypes.float8_e5m2

EPS = 1e-5
NB, CIN, H, W = 32, 256, 56, 56
PLANES, OUTP = 128, 512
NCORES = 8
NPC = NB // NCORES              # samples per core
HW1 = H * W                     # 3136
WP = W + 2                      # 58 (padded row length)
HP = H + 2
HO = WO = 28
HWO = HO * WO                   # 784
PT1 = 392                       # conv1 pixel tile = 7 rows of 56
NPT1 = HW1 // PT1               # 8
PT3 = 392                       # stage2/3 pixel tile = 14 out rows of 28
SCALE = 1024.0                  # x pre-scale (power of two)

# DMA chunk plan (pixels): fine for sample 0 so conv1 starts early
CHUNKS = {0: [(0, PT1), (PT1, PT1), (2 * PT1, 2 * PT1), (4 * PT1, 4 * PT1)]}
for _n in range(1, NPC):
    CHUNKS[_n] = [(0, 4 * PT1), (4 * PT1, 4 * PT1)]

# wts16 column layout (fp16, 128 partitions = contraction dim)
_B1 = 0            # 2 ktiles x 128: conv1 hi +-1
_S3 = 256          # 4 oc x 128: conv3 +-1
_SCHI = 768        # 2 ktiles x 512: shortcut hi +-2^-10
_W16COLS = 1792

# wts8 column layout (fp8e4)
_B2P = 0           # 4 x [2 x 128]: conv2 tap pairs
_B2S = 1024        # 128: conv2 single tap (2,2)
_B1L = 1152        # [2 x 128]: conv1 lo +-1 (kt pair)
_W8COLS = 1408

# wsc8 (fp8e5): 4 oc x [2 kt x 128]: shortcut lo +-2^-10
_WSCCOLS = 1024

# bias cols ([128, 10] f32):
#   0: (c1/s1)*SCALE   1: c2/s2   2+oc: q' = ssc/s3   6+oc: c'' = (c3+csc)/s3

# conv2 tap pairs (dy, dx): three same-row pairs + one same-col pair + single
PAIRS = [((0, 0), (0, 1)), ((1, 0), (1, 1)), ((2, 0), (2, 1)), ((0, 2), (1, 2))]
SINGLE = (2, 2)


def build_bass():
    nc = bacc.Bacc("TRN2", target_bir_lowering=False, debug=False)
    nxh = NPC * 2 * 128 * HW1
    xhi_d = nc.dram_tensor("xhi", [nxh], F16, kind="ExternalInput")
    xlo8_d = nc.dram_tensor("xlo8", [nxh], E4, kind="ExternalInput")
    w16_d = nc.dram_tensor("w16", [128, _W16COLS], F16, kind="ExternalInput")
    w8_d = nc.dram_tensor("w8", [128, _W8COLS], E4, kind="ExternalInput")
    wsc8_d = nc.dram_tensor("wsc8", [128, _WSCCOLS], E5, kind="ExternalInput")
    bias_d = nc.dram_tensor("bias", [128, 10], F32, kind="ExternalInput")
    out_d = nc.dram_tensor("out", [NPC * 4 * 2 * 128, PT3], E4, kind="ExternalOutput")
    warm_d = nc.dram_tensor("warm", [128, 8], F32, kind="ExternalOutput")

    with tile.TileContext(nc) as tc:
        import contextlib

        with contextlib.ExitStack() as ctx:
            const = ctx.enter_context(tc.tile_pool(name="const", bufs=1))
            xpool = ctx.enter_context(tc.tile_pool(name="x", bufs=1))
            ypool = ctx.enter_context(tc.tile_pool(name="y", bufs=1))
            opool = ctx.enter_context(tc.tile_pool(name="o", bufs=6))
            upool = ctx.enter_context(tc.tile_pool(name="u", bufs=4))
            p1pool = ctx.enter_context(tc.tile_pool(name="p1", bufs=2, space="PSUM"))
            p2pool = ctx.enter_context(tc.tile_pool(name="p2", bufs=2, space="PSUM"))
            pscpool = ctx.enter_context(tc.tile_pool(name="psc", bufs=2, space="PSUM"))
            ps3pool = ctx.enter_context(tc.tile_pool(name="ps3", bufs=2, space="PSUM"))

            # conv1's fp16 weight block ships first on sync; the rest of the
            # fp16 weights ride scalar once, ahead of all ACTs
            w16 = const.tile([128, _W16COLS], F16, tag="w16")
            bias = const.tile([128, 10], F32, tag="bias")
            w8 = const.tile([128, _W8COLS], E4, tag="w8")
            wsc8 = const.tile([128, _WSCCOLS], E5, tag="wsc8")

            # PE prewarm in a stage-2 PSUM bank (stage-1 banks stay free for
            # the first real matmuls).  The input tile is never written -
            # garbage values are fine and skipping the memset removes the
            # cross-engine dependency, so the PE starts the moment its queue
            # opens.  Escape chain prevents DCE.
            warm = const.tile([128, 512], F16, tag="warm")
            nc.vector.memset(warm[:], 0.0)
            for r in range(8):
                pw = p2pool.tile([128, 512], F32, tag="p2", name=f"warm{r}")
                nc.tensor.matmul(
                    pw[:], warm[:, 0:128], warm[:], start=True, stop=True
                )
            for r in range(24):
                pw = p2pool.tile([128, 512], F32, tag="p2", name=f"warmb{r}")
                nc.tensor.matmul(
                    pw[:, 0:128], warm[:, 0:128], warm[:, 0:128],
                    start=True, stop=True,
                )
            wout = const.tile([128, 8], F32, tag="wout")
            nc.vector.tensor_copy(wout[:], pw[:, 0:8])

            xhi = {}
            xlo8 = {}
            for n in range(NPC):
                for kt in range(2):
                    xhi[n, kt] = xpool.tile(
                        [128, HW1], F16, tag=f"xhi{n}{kt}", name=f"xhi{n}{kt}"
                    )
                xlo8[n] = xpool.tile(
                    [128, 2 * HW1], E4, tag=f"xlo8{n}", name=f"xlo8{n}"
                )

            # x DMAs: DRAM is chunk-contiguous in emission order.  The very
            # first chunk's three pieces land in parallel on sync/scalar/
            # gpsimd; everything after streams on sync (which carries nothing
            # else until the tail).  w16's conv1 block follows immediately;
            # the rest of w16 rides scalar once, ahead of all ACTs.
            offh = 0
            offl = 0
            first = True
            for n in range(NPC):
                for p0, w in CHUNKS[n]:
                    for kt in range(2):
                        span = 128 * w
                        src_hi = xhi_d.ap()[offh : offh + span].rearrange(
                            "(p w) -> p w", w=w
                        )
                        eng = nc.scalar if (first and kt == 1) else nc.sync
                        eng.dma_start(xhi[n, kt][:, p0 : p0 + w], src_hi)
                        offh += span
                    span = 128 * 2 * w
                    src_lo = xlo8_d.ap()[offl : offl + span].rearrange(
                        "(p w) -> p w", w=2 * w
                    )
                    (nc.scalar if first else nc.sync).dma_start(
                        xlo8[n][:, 2 * p0 : 2 * (p0 + w)], src_lo
                    )
                    offl += span
                    if first:
                        nc.sync.dma_start(w16[:, 0:256], w16_d.ap()[:, 0:256])
                        nc.scalar.dma_start(
                            w16[:, 256:_W16COLS], w16_d.ap()[:, 256:_W16COLS]
                        )
                        first = False
            nc.gpsimd.dma_start(bias[:], bias_d.ap())
            nc.gpsimd.dma_start(w8[:], w8_d.ap())
            nc.gpsimd.dma_start(wsc8[:], wsc8_d.ap())
            nc.gpsimd.dma_start(warm_d.ap(), wout[:])

            y1 = {}
            y2 = {}
            for n in range(NPC):
                y1[n] = ypool.tile([128, HP * WP], E4, tag=f"y1_{n}", name=f"y1_{n}")
                y2[n] = ypool.tile([128, HWO], E4, tag=f"y2_{n}", name=f"y2_{n}")

            for n in range(NPC):
                v1 = y1[n][:].rearrange("p (h w) -> p h w", w=WP)
                nc.vector.memset(v1[:, 0:1, :], 0.0)
                nc.vector.memset(v1[:, HP - 1 : HP, :], 0.0)
                nc.vector.memset(v1[:, 1 : HP - 1, 0:1], 0.0)
                nc.vector.memset(v1[:, 1 : HP - 1, WP - 1 : WP], 0.0)

            def stage1(n, pts=range(NPT1)):
                v1 = y1[n][:].rearrange("p (h w) -> p h w", w=WP)
                for pt in pts:
                    p1 = p1pool.tile([128, PT1], F32, tag="p1")
                    ps = slice(pt * PT1, (pt + 1) * PT1)
                    for kt in range(2):
                        nc.tensor.matmul(
                            p1[:],
                            w16[:, _B1 + kt * 128 : _B1 + kt * 128 + 128],
                            xhi[n, kt][:, ps],
                            start=(kt == 0),
                            stop=False,
                        )
                    lo_rhs = (
                        xlo8[n][:, 2 * pt * PT1 : 2 * (pt + 1) * PT1]
                        .rearrange("p (w two) -> p two w", two=2)
                    )
                    nc.tensor.matmul(
                        p1[:],
                        w8[:, _B1L : _B1L + 256].rearrange(
                            "p (two m) -> p two m", two=2
                        ),
                        lo_rhs,
                        start=False,
                        stop=True,
                        perf_mode=DRM,
                    )
                    nc.scalar.activation(
                        v1[:, 7 * pt + 1 : 7 * pt + 8, 1 : 1 + W],
                        p1[:].rearrange("p (h w) -> p h w", w=W),
                        AF.Sign,
                        bias=bias[:, 0:1],
                        scale=1.0,
                    )

            def stage2(n, ht):
                v1 = y1[n][:].rearrange("p (h w) -> p h w", w=WP)
                p2 = p2pool.tile([128, PT3], F32, tag="p2", name=f"p2_{n}_{ht}")
                for i, ((dy0, dx0), (dy1, dx1)) in enumerate(PAIRS):
                    if dy0 == dy1:
                        # same row: adjacent-byte pairs (dx0, dx0+1)
                        rows = v1[:, 28 * ht + dy0 : 28 * ht + dy0 + 28 : 2, :]
                        pair = rows[:, :, dx0 : dx0 + 56].rearrange(
                            "p h (w two) -> p two h w", two=2
                        )
                    else:
                        # same col: adjacent-row pairs (dy0, dy0+1)
                        rows = v1[:, 28 * ht + dy0 : 28 * ht + dy0 + 28, :]
                        pair = rows.rearrange("p (h two) w -> p two h w", two=2)[
                            :, :, :, dx0 : dx0 + 56 : 2
                        ]
                    nc.tensor.matmul(
                        p2[:],
                        w8[:, _B2P + i * 256 : _B2P + (i + 1) * 256].rearrange(
                            "p (two m) -> p two m", two=2
                        ),
                        pair,
                        start=(i == 0),
                        stop=False,
                        perf_mode=DRM,
                    )
                dy, dx = SINGLE
                nc.tensor.matmul(
                    p2[:],
                    w8[:, _B2S : _B2S + 128],
                    v1[:, 28 * ht + dy : 28 * ht + dy + 28 : 2, dx : dx + 56 : 2],
                    start=False,
                    stop=True,
                )
                nc.scalar.activation(
                    y2[n][:, ht * PT3 : (ht + 1) * PT3],
                    p2[:],
                    AF.Sign,
                    bias=bias[:, 1:2],
                    scale=1.0,
                )

            def stage3(n, ht):
                yslice = y2[n][:, ht * PT3 : (ht + 1) * PT3]
                # shortcut lo: stride-2 quadrant of the full fp8 plane,
                # kt byte-pairs: [p, 2(s=1), 14 rows, 28 cols]
                vlo = xlo8[n][:].rearrange("p (h w two) -> p two h w", w=W, two=2)
                lo_rhs = vlo[:, :, 28 * ht : 28 * ht + 28 : 2, 0:56:2]
                for oc in range(4):
                    psc = pscpool.tile([128, PT3], F32, tag="psc")
                    for kt in range(2):
                        rhs = (
                            xhi[n, kt][:]
                            .rearrange("p (h w) -> p h w", w=W)
                            [:, 28 * ht : 28 * ht + 28 : 2, 0:56:2]
                        )
                        nc.tensor.matmul(
                            psc[:],
                            w16[
                                :,
                                _SCHI + kt * 512 + oc * 128 : _SCHI
                                + kt * 512
                                + oc * 128
                                + 128,
                            ],
                            rhs,
                            start=(kt == 0),
                            stop=False,
                        )
                    nc.tensor.matmul(
                        psc[:],
                        wsc8[:, oc * 256 : (oc + 1) * 256].rearrange(
                            "p (two m) -> p two m", two=2
                        ),
                        lo_rhs,
                        start=False,
                        stop=True,
                        perf_mode=DRM,
                    )
                    ps3 = ps3pool.tile([128, PT3], F32, tag="ps3")
                    nc.tensor.matmul(
                        ps3[:],
                        w16[:, _S3 + oc * 128 : _S3 + oc * 128 + 128],
                        yslice,
                        start=True,
                        stop=True,
                    )
                    u = upool.tile([128, PT3], F32, tag="u")
                    nc.vector.tensor_scalar(
                        u[:],
                        psc[:],
                        bias[:, 2 + oc : 3 + oc],
                        bias[:, 6 + oc : 7 + oc],
                        ALU.mult,
                        ALU.add,
                    )
                    nc.vector.tensor_tensor(u[:], u[:], ps3[:], ALU.add)
                    ot = opool.tile([128, PT3], E4, tag="ot")
                    nc.scalar.activation(ot[:], u[:], AF.Sign, bias=0.0, scale=1.0)
                    (nc.sync if n >= 2 else nc.gpsimd).dma_start(
                        out_d.ap()[
                            ((n * 4 + oc) * 2 + ht) * 128 : ((n * 4 + oc) * 2 + ht + 1)
                            * 128,
                            :,
                        ],
                        ot[:],
                    )

            for n in range(NPC):
                if n == 0:
                    stage1(n, range(5))
                    stage2(n, 0)
                    stage1(n, range(5, NPT1))
                    stage2(n, 1)
                else:
                    stage1(n)
                    stage2(n, 0)
                    stage2(n, 1)
                stage3(n, 0)
                stage3(n, 1)

    nc.compile()
    return nc


def _prep_inputs(x, W1, W2, W3, Wsc, g1, b1, m1, v1, g2, b2, m2, v2,
                 g3, b3, m3, v3, gs, bs, ms, vs):
    f32 = np.float32

    def sgn(w):
        return np.where(w >= 0, 1.0, -1.0).astype(f32)

    def fold(g, b, m, v):
        s = (g / np.sqrt(v + EPS)).astype(f32)
        return s, (b - m * s).astype(f32)

    s1, c1 = fold(g1, b1, m1, v1)
    s2, c2 = fold(g2, b2, m2, v2)
    s3, c3 = fold(g3, b3, m3, v3)
    ssc, csc = fold(gs, bs, ms, vs)

    w16 = np.zeros((128, _W16COLS), np.float16)
    b1t = sgn(W1[:, :, 0, 0]).T                     # [256, 128]
    w16[:, _B1 : _B1 + 128] = b1t[:128]
    w16[:, _B1 + 128 : _B1 + 256] = b1t[128:]
    w3t = sgn(W3[:, :, 0, 0]).T                     # [128, 512]
    w16[:, _S3 : _S3 + 512] = w3t
    wsct = sgn(Wsc[:, :, 0, 0]).T * f32(1.0 / SCALE)  # [256, 512], +-2^-10
    w16[:, _SCHI : _SCHI + 512] = wsct[:128]
    w16[:, _SCHI + 512 : _SCHI + 1024] = wsct[128:]

    w8 = np.zeros((128, _W8COLS), E4NP)
    b2v = sgn(W2)                                   # [128, 128, 3, 3]
    for i, ((dy0, dx0), (dy1, dx1)) in enumerate(PAIRS):
        w8[:, _B2P + i * 256 : _B2P + i * 256 + 128] = b2v[:, :, dy0, dx0].T.astype(
            E4NP
        )
        w8[:, _B2P + i * 256 + 128 : _B2P + (i + 1) * 256] = (
            b2v[:, :, dy1, dx1].T.astype(E4NP)
        )
    w8[:, _B2S : _B2S + 128] = b2v[:, :, SINGLE[0], SINGLE[1]].T.astype(E4NP)
    w8[:, _B1L : _B1L + 128] = b1t[:128].astype(E4NP)
    w8[:, _B1L + 128 : _B1L + 256] = b1t[128:].astype(E4NP)

    wsc8 = np.zeros((128, _WSCCOLS), E5NP)
    for oc in range(4):
        for kt in range(2):
            blk = wsct[kt * 128 : (kt + 1) * 128, oc * 128 : (oc + 1) * 128]
            wsc8[:, oc * 256 + kt * 128 : oc * 256 + (kt + 1) * 128] = blk.astype(
                E5NP
            )

    bias = np.zeros((128, 10), f32)
    bias[:, 0] = (c1 / s1) * f32(SCALE)
    bias[:, 1] = c2 / s2
    bias[:, 2:6] = (ssc / s3).reshape(4, 128).T
    bias[:, 6:10] = ((c3 + csc) / s3).reshape(4, 128).T

    xs = (x.astype(f32) * f32(SCALE)).reshape(NB, 2, 128, HW1)
    xhi = xs.astype(np.float16)
    xlo_f = xs - xhi.astype(f32)
    # kt byte-pairs: [NB, 128, HW1, 2]
    xlo8 = xlo_f.transpose(0, 2, 3, 1).astype(E4NP)

    # chunk-contiguous per-core flat layout matching build_bass emission order
    xhic = []
    xloc = []
    for c in range(NCORES):
        ph = []
        pl = []
        for n in range(NPC):
            gh = xhi[c * NPC + n]              # [2, 128, HW1]
            gl = xlo8[c * NPC + n]             # [128, HW1, 2]
            for p0, w in CHUNKS[n]:
                for kt in range(2):
                    ph.append(gh[kt, :, p0 : p0 + w].reshape(-1))
                pl.append(gl[:, p0 : p0 + w, :].reshape(-1))
        xhic.append(np.concatenate(ph))
        xloc.append(np.concatenate(pl))
    return xhic, xloc, w16, w8, wsc8, bias


_NC_CACHE = []


def _assemble(res_results):
    outs = []
    for r in res_results:
        o = np.asarray(r["out"]).astype(np.float32).reshape(NPC, 4, 2, 128, PT3)
        o = o.transpose(0, 1, 3, 2, 4).reshape(NPC, OUTP, HO, WO)
        outs.append(o)
    return np.concatenate(outs, axis=0).astype(np.float32)


def make_in_maps(inputs):
    xhi, xlo8, w16, w8, wsc8, bias = _prep_inputs(**inputs)
    return [
        {
            "xhi": xhi[c],
            "xlo8": xlo8[c],
            "w16": w16,
            "w8": w8,
            "wsc8": wsc8,
            "bias": bias,
        }
        for c in range(NCORES)
    ]


def kernel(**inputs):
    inputs = {k: np.asarray(v) for k, v in inputs.items()}
    in_maps = make_in_maps(inputs)
    if not _NC_CACHE:
        _NC_CACHE.append(build_bass())
    nc = _NC_CACHE[0]
    res = run_bass_kernel_spmd(nc, in_maps, core_ids=list(range(NCORES)))
    return _assemble(res.results)
